# revision 3
# baseline (speedup 1.0000x reference)
"""Causal self-attention (B=2, S=2048, D=1024, H=16) on 8 Trainium2 NeuronCores.

Sharding: core c handles batch b = c//4 and head-group g = c%4 (4 heads, 256
channels). Per-core device program (identical NEFF on all cores, inputs differ):

  1. x[b] is DMA-cast f32->bf16 to a DRAM bounce, then xbar-transpose-loaded
     into SBUF as xT [D=8x128, S] (the TensorEngine contracts over the
     partition axis, so activations must be D-major).
  2. QKV projections produce qT/kT [256, S] (head-dim on partitions) and
     v1 [S, 260] (natural orientation, with a ones-column per head appended
     via the bias row so the PV matmul also yields softmax denominators).
     The softmax 1/sqrt(D) scale is folded into Wq/bq on the host.
  3. Attention per head: logitsT[t, s] tiles come from a single K=64 matmul
     (two heads packed in the PE array via row tile_position), exp runs on
     the Scalar engine straight out of PSUM, causal masking is a bf16
     triangle multiply on diagonal tiles only, and PV accumulates
     zT_aug[65, s] = [v.T @ expT ; sum_t expT].  Row 64 is the softmax
     denominator; normalization is a reciprocal + partition_broadcast +
     vector multiply.
  4. z (256 rows per core) is AllGathered across the 4 cores of the batch
     group to zT_full [1024, S]; each core then computes its 256 output
     channels: out[:, g*256:(g+1)*256] = z @ Wo.T[:, slice].
Host-side: slice/scale weights per core, run SPMD, concatenate outputs.
"""

import numpy as np

EMBED_DIM = 1024
NUM_HEADS = 16
HEAD_DIM = 64
BATCH = 2
N_CORES = 8
CORES_PER_BATCH = 4
HEADS_PER_CORE = 4
DQ = HEADS_PER_CORE * HEAD_DIM  # 256 q/k/v channels per core
VW = HEAD_DIM + 1  # v block width incl. ones column
DV1 = HEADS_PER_CORE * VW  # 260
P = 128

_NC_CACHE = {}


def _build_nc(seq):
    import concourse.bass as bass  # noqa: F401
    import concourse.mybir as mybir
    import concourse.tile as tile
    from concourse import bacc

    fp32 = mybir.dt.float32
    bf16 = mybir.dt.bfloat16
    AF = mybir.ActivationFunctionType
    ALU = mybir.AluOpType

    S = seq
    SC = 512  # s-chunk width
    NSC = S // SC  # s-chunks
    NT = S // P  # t-tiles
    ND = EMBED_DIM // P  # D-tiles (8)
    TPC = SC // P  # t-tiles per s-chunk (4)

    nc = bacc.Bacc("TRN2", target_bir_lowering=False, num_devices=N_CORES)

    x = nc.declare_dram_parameter("x", [S, EMBED_DIM], fp32, isOutput=False)
    wq = nc.declare_dram_parameter("wq", [EMBED_DIM, DQ], fp32, isOutput=False)
    bq = nc.declare_dram_parameter("bq", [DQ], fp32, isOutput=False)
    wk = nc.declare_dram_parameter("wk", [EMBED_DIM, DQ], fp32, isOutput=False)
    bk = nc.declare_dram_parameter("bk", [DQ], fp32, isOutput=False)
    wv1 = nc.declare_dram_parameter("wv1", [EMBED_DIM, DV1], fp32, isOutput=False)
    bv1 = nc.declare_dram_parameter("bv1", [DV1], fp32, isOutput=False)
    wot = nc.declare_dram_parameter("wot", [EMBED_DIM, DQ], fp32, isOutput=False)
    out = nc.declare_dram_parameter("out", [S, DQ], fp32, isOutput=True)

    with tile.TileContext(nc) as tc:
        with (
            tc.tile_pool(name="const", bufs=1) as constp,
            tc.tile_pool(name="dram", bufs=1, space="DRAM") as dram,
            tc.tile_pool(name="big", bufs=1) as big,
            tc.tile_pool(name="exp", bufs=24) as expp,
            tc.tile_pool(name="small", bufs=4) as small,
            tc.tile_pool(name="outsb", bufs=4) as outsb,
            tc.tile_pool(name="psA", bufs=2, space="PSUM") as psA,
            tc.tile_pool(name="psLG", bufs=4, space="PSUM") as psLG,
            tc.tile_pool(name="psZ", bufs=2, space="PSUM") as psZ,
        ):
            # ---- constants / weights -------------------------------------
            wq_sb = big.tile([P, ND, DQ], bf16, name="wq_sb")
            wk_sb = big.tile([P, ND, DQ], bf16, name="wk_sb")
            wv1_sb = big.tile([P, ND, DV1], bf16, name="wv1_sb")
            wot_sb = big.tile([P, ND, DQ], bf16, name="wot_sb")
            nc.gpsimd.dma_start(wq_sb[:], wq.rearrange("(o p) n -> p o n", p=P))
            nc.gpsimd.dma_start(wk_sb[:], wk.rearrange("(o p) n -> p o n", p=P))
            nc.gpsimd.dma_start(wv1_sb[:], wv1.rearrange("(o p) n -> p o n", p=P))
            nc.gpsimd.dma_start(wot_sb[:], wot.rearrange("(o p) n -> p o n", p=P))

            bq_sb = constp.tile([P, DQ // P], fp32, name="bq_sb")
            bk_sb = constp.tile([P, DQ // P], fp32, name="bk_sb")
            nc.sync.dma_start(bq_sb[:], bq.rearrange("(o p) -> p o", p=P))
            nc.sync.dma_start(bk_sb[:], bk.rearrange("(o p) -> p o", p=P))
            bv1_bf = constp.tile([1, DV1], bf16, name="bv1_bf")
            nc.gpsimd.dma_start(bv1_bf[:], bv1[None, :])
            ones_bf = constp.tile([1, P], bf16, name="ones_bf")
            nc.gpsimd.memset(ones_bf[:], 1.0)

            # causal triangle mask (keep where t_local <= s_local)
            mask_f = constp.tile([P, P], fp32, name="mask_f")
            mask_bf = constp.tile([P, P], bf16, name="mask_bf")
            nc.gpsimd.memset(mask_f[:], 0.0)
            nc.gpsimd.affine_select(
                out=mask_f[:],
                in_=mask_f[:],
                compare_op=ALU.is_gt,  # iota > 0 ? keep in_ (0.0) : fill (1.0)
                fill=1.0,
                base=0,
                pattern=[[-1, P]],  # iota[p, f] = p - f;  p<=f -> fill=1.0
                channel_multiplier=1,
            )
            nc.vector.tensor_copy(mask_bf[:], mask_f[:])

            # ---- x transpose + QKV projections, per s-chunk --------------
            xT = big.tile([P, ND, S], bf16, name="xT")
            qT = big.tile([P, DQ // P, S], bf16, name="qT")
            kT = big.tile([P, DQ // P, S], bf16, name="kT")
            v1 = big.tile([P, NT, DV1], bf16, name="v1")

            for c in range(NSC):
                xbf = dram.tile([SC, EMBED_DIM], bf16, name=f"xbf_{c}")
                nc.gpsimd.dma_start(xbf[:], x[c * SC : (c + 1) * SC, :])
                for d in range(ND):
                    nc.sync.dma_start(
                        xT[:, d, c * SC : (c + 1) * SC],
                        xbf[:, d * P : (d + 1) * P],
                        transpose=True,
                    )
                # qT / kT for this s-chunk
                for w_sb, b_sb, dstT in ((wq_sb, bq_sb, qT), (wk_sb, bk_sb, kT)):
                    for j in range(DQ // P):
                        ps = psA.tile([P, SC], fp32, name="mmps")
                        for d in range(ND):
                            nc.tensor.matmul(
                                ps[:],
                                w_sb[:, d, j * P : (j + 1) * P],
                                xT[:, d, c * SC : (c + 1) * SC],
                                start=(d == 0),
                                stop=(d == ND - 1),
                            )
                        nc.vector.tensor_scalar(
                            dstT[:, j, c * SC : (c + 1) * SC],
                            ps[:],
                            b_sb[:, j : j + 1],
                            None,
                            ALU.add,
                        )
                # v1 (natural orientation) for this chunk's t-tiles
                for tt in range(c * TPC, (c + 1) * TPC):
                    ps = psA.tile([P, SC], fp32, name="mmps")[:, :DV1]
                    for d in range(ND):
                        nc.tensor.matmul(
                            ps[:],
                            xT[:, d, tt * P : (tt + 1) * P],
                            wv1_sb[:, d, :],
                            start=(d == 0),
                            stop=False,
                        )
                    nc.tensor.matmul(
                        ps[:], ones_bf[:], bv1_bf[:], start=False, stop=True
                    )
                    nc.vector.tensor_copy(v1[:, tt, :], ps[:])

            # ---- attention -----------------------------------------------
            zsb = [
                big.tile([HEAD_DIM, S], bf16, name=f"zsb_{h}")
                for h in range(HEADS_PER_CORE)
            ]
            for hp in range(HEADS_PER_CORE // 2):  # head pair = dq-tile index
                for c in range(NSC):
                    zps = [
                        psZ.tile([VW, SC], fp32, name="zps") for _ in range(2)
                    ]
                    ntt = (c + 1) * TPC
                    for tt in range(ntt):
                        off = max(0, (tt - c * TPC)) * P
                        n = SC - off
                        lg = [
                            psLG.tile([P, SC], fp32, name="lgps") for _ in range(2)
                        ]
                        ex = [
                            expp.tile([P, SC], bf16, name="expt") for _ in range(2)
                        ]
                        for i in range(2):
                            p0 = 64 * i
                            nc.tensor.matmul(
                                lg[i][:, off:],
                                kT[p0 : p0 + 64, hp, tt * P : (tt + 1) * P],
                                qT[p0 : p0 + 64, hp, c * SC + off : (c + 1) * SC],
                                start=True,
                                stop=True,
                                tile_position=(p0, 0),
                            )
                            nc.scalar.activation(
                                ex[i][:, off:], lg[i][:, off:], AF.Exp
                            )
                            if tt >= c * TPC:  # diagonal tile: causal triangle
                                nc.vector.tensor_tensor(
                                    ex[i][:, off : off + P],
                                    ex[i][:, off : off + P],
                                    mask_bf[:],
                                    ALU.mult,
                                )
                            h = 2 * hp + i
                            nc.tensor.matmul(
                                zps[i][:, off:],
                                v1[:, tt, h * VW : (h + 1) * VW],
                                ex[i][:, off:],
                                start=(tt == 0),
                                stop=(tt == ntt - 1),
                            )
                    for i in range(2):
                        h = 2 * hp + i
                        recip = small.tile([1, SC], fp32, name="recip")
                        rb = small.tile([HEAD_DIM, SC], fp32, name="recip_bc")
                        nc.vector.reciprocal(recip[:], zps[i][VW - 1 : VW, :])
                        nc.gpsimd.partition_broadcast(rb[:], recip[:])
                        nc.vector.tensor_tensor(
                            zsb[h][:, c * SC : (c + 1) * SC],
                            zps[i][:HEAD_DIM, :],
                            rb[:],
                            ALU.mult,
                        )

            # ---- AllGather z over the 4-core batch group -----------------
            z_loc = dram.tile([DQ, S], bf16, name="z_loc")
            z_full = dram.tile([CORES_PER_BATCH * DQ, S], bf16, name="z_full")
            for h in range(HEADS_PER_CORE):
                nc.sync.dma_start(
                    z_loc[h * HEAD_DIM : (h + 1) * HEAD_DIM, :], zsb[h][:]
                )
            nc.gpsimd.collective_compute(
                "AllGather",
                ALU.bypass,
                replica_groups=[
                    list(range(g * CORES_PER_BATCH, (g + 1) * CORES_PER_BATCH))
                    for g in range(N_CORES // CORES_PER_BATCH)
                ],
                ins=[z_loc.opt()],
                outs=[z_full.opt()],
            )
            zf = big.tile([P, ND, S], bf16, name="zf")
            nc.sync.dma_start(zf[:], z_full.rearrange("(o p) n -> p o n", p=P))

            # ---- output projection ---------------------------------------
            for i in range(NT):
                ps = psA.tile([P, SC], fp32, name="mmps")[:, :DQ]
                for d in range(ND):
                    nc.tensor.matmul(
                        ps[:],
                        zf[:, d, i * P : (i + 1) * P],
                        wot_sb[:, d, :],
                        start=(d == 0),
                        stop=(d == ND - 1),
                    )
                osb = outsb.tile([P, DQ], fp32, name="osb")
                nc.vector.tensor_copy(osb[:], ps[:])
                nc.sync.dma_start(out[i * P : (i + 1) * P, :], osb[:])

    nc.finalize()
    return nc


def _get_nc(seq):
    if seq not in _NC_CACHE:
        _NC_CACHE[seq] = _build_nc(seq)
    return _NC_CACHE[seq]


def shard_inputs(x, Wq, bq, Wk, bk, Wv, bv, Wo):
    """Build the 8 per-core input maps (host-side sharding)."""
    scale = 1.0 / np.sqrt(np.float32(EMBED_DIM))
    x = np.asarray(x, np.float32)
    in_maps = []
    for c in range(N_CORES):
        b, g = divmod(c, CORES_PER_BATCH)
        sl = slice(g * DQ, (g + 1) * DQ)
        wv1 = np.zeros((EMBED_DIM, DV1), np.float32)
        bv1 = np.zeros((DV1,), np.float32)
        for h in range(HEADS_PER_CORE):
            col = g * DQ + h * HEAD_DIM
            wv1[:, h * VW : h * VW + HEAD_DIM] = Wv[:, col : col + HEAD_DIM]
            bv1[h * VW : h * VW + HEAD_DIM] = bv[col : col + HEAD_DIM]
            bv1[h * VW + HEAD_DIM] = 1.0
        in_maps.append(
            {
                "x": np.ascontiguousarray(x[b]),
                "wq": np.ascontiguousarray(Wq[:, sl]) * scale,
                "bq": np.ascontiguousarray(bq[sl]) * scale,
                "wk": np.ascontiguousarray(Wk[:, sl]),
                "bk": np.ascontiguousarray(bk[sl]),
                "wv1": wv1,
                "bv1": bv1,
                "wot": np.ascontiguousarray(Wo[sl, :].T),
            }
        )
    return in_maps


def kernel(x, Wq, bq, Wk, bk, Wv, bv, Wo):
    from concourse.bass_utils import run_bass_kernel_spmd

    x = np.asarray(x, np.float32)
    B, S, D = x.shape
    nc = _get_nc(S)
    in_maps = shard_inputs(x, Wq, bq, Wk, bk, Wv, bv, Wo)
    res = run_bass_kernel_spmd(nc, in_maps, core_ids=list(range(N_CORES)))
    out = np.empty((B, S, D), np.float32)
    for c in range(N_CORES):
        b, g = divmod(c, CORES_PER_BATCH)
        out[b, :, g * DQ : (g + 1) * DQ] = res.results[c]["out"]
    return out


# revision 4
# speedup vs baseline: 1.9427x; 1.9427x over previous
"""Causal self-attention (B=2, S=2048, D=1024, H=16) on 8 Trainium2 NeuronCores.

Sharding: core c handles batch b = c//4 and head-group g = c%4 (4 heads, 256
channels).  Per-core device program (identical NEFF on all cores):

  1. x[b] is DMA-cast f32->bf16 to a DRAM bounce per 512-row s-chunk, then
     xbar-transpose-loaded into SBUF as xT [D=8x128, S] (TensorE contracts
     over the partition axis, so activations must be D-major).
  2. QKV projections produce qT/kT [256, S] (head-dim on partitions) and
     v1 [S, 260] (natural orientation, with a ones-column per head appended
     via the bias row so the PV matmul also yields softmax denominators).
     The softmax 1/sqrt(D) scale is folded into Wq/bq on the host.
  3. Attention per head pair: logitsT[t, s] tiles from K=64 matmuls with two
     heads packed in the PE array via row tile_position into one 2-bank PSUM
     tile, exp on ScalarE straight out of PSUM, causal triangle mask as a
     bf16 multiply on diagonal tiles, PV accumulates
     zT_aug[65, s] = [v.T @ expT ; sum_t expT] (row 64 = denominator).
     The t-loop is emitted in groups (all QK/exp of a group, then PV) so the
     in-order PE never waits on ScalarE; projection work for the next s-chunk
     is interleaved between groups to fill PE slack.
  4. Each core computes its full-width partial output
     out_partial = z_local @ Wo.T[local 256 rows, :], pipelined per s-chunk.
     The host sums the 4 partials per batch (the row-parallel reduction).
"""

import numpy as np

EMBED_DIM = 1024
NUM_HEADS = 16
HEAD_DIM = 64
BATCH = 2
N_CORES = 8
CORES_PER_BATCH = 4
HEADS_PER_CORE = 4
DQ = HEADS_PER_CORE * HEAD_DIM  # 256 q/k/v channels per core
VW = HEAD_DIM + 1  # v block width incl. ones column
DV1 = HEADS_PER_CORE * VW  # 260
P = 128

_NC_CACHE = {}


def _build_nc(seq):
    import concourse.bass as bass  # noqa: F401
    import concourse.mybir as mybir
    import concourse.tile as tile
    from concourse import bacc

    fp32 = mybir.dt.float32
    bf16 = mybir.dt.bfloat16
    AF = mybir.ActivationFunctionType
    ALU = mybir.AluOpType

    S = seq
    SC = 512  # s-chunk width
    NSC = S // SC  # s-chunks
    NT = S // P  # t-tiles
    ND = EMBED_DIM // P  # D-tiles (8)
    TPC = SC // P  # t-tiles per s-chunk (4)
    TGROUP = 8  # attention t-loop software-pipeline group

    nc = bacc.Bacc("TRN2", target_bir_lowering=False, num_devices=N_CORES)

    x = nc.declare_dram_parameter("x", [S, EMBED_DIM], fp32, isOutput=False)
    wq = nc.declare_dram_parameter("wq", [EMBED_DIM, DQ], fp32, isOutput=False)
    bq = nc.declare_dram_parameter("bq", [DQ], fp32, isOutput=False)
    wk = nc.declare_dram_parameter("wk", [EMBED_DIM, DQ], fp32, isOutput=False)
    bk = nc.declare_dram_parameter("bk", [DQ], fp32, isOutput=False)
    wv1 = nc.declare_dram_parameter("wv1", [EMBED_DIM, DV1], fp32, isOutput=False)
    bv1 = nc.declare_dram_parameter("bv1", [DV1], fp32, isOutput=False)
    wot = nc.declare_dram_parameter("wot", [DQ, EMBED_DIM], fp32, isOutput=False)
    out = nc.declare_dram_parameter("out", [S, EMBED_DIM], fp32, isOutput=True)

    with tile.TileContext(nc) as tc:
        with (
            tc.tile_pool(name="const", bufs=1) as constp,
            tc.tile_pool(name="dram", bufs=1, space="DRAM") as dram,
            tc.tile_pool(name="big", bufs=1) as big,
            tc.tile_pool(name="exp", bufs=12) as expp,
            tc.tile_pool(name="small", bufs=4) as small,
            tc.tile_pool(name="outsb", bufs=3) as outsb,
            tc.tile_pool(name="psA", bufs=2, space="PSUM") as psA,
            tc.tile_pool(name="psLG", bufs=2, space="PSUM") as psLG,
            tc.tile_pool(name="psZ", bufs=2, space="PSUM") as psZ,
        ):
            # ---- constants / weights -------------------------------------
            wq_sb = big.tile([P, ND, DQ], bf16, name="wq_sb")
            wk_sb = big.tile([P, ND, DQ], bf16, name="wk_sb")
            wv1_sb = big.tile([P, ND, DV1], bf16, name="wv1_sb")
            wot_sb = big.tile([P, DQ // P, EMBED_DIM], bf16, name="wot_sb")
            nc.gpsimd.dma_start(wq_sb[:], wq.rearrange("(o p) n -> p o n", p=P))
            nc.gpsimd.dma_start(wk_sb[:], wk.rearrange("(o p) n -> p o n", p=P))
            nc.gpsimd.dma_start(wv1_sb[:], wv1.rearrange("(o p) n -> p o n", p=P))
            nc.gpsimd.dma_start(wot_sb[:], wot.rearrange("(o p) n -> p o n", p=P))

            bq_sb = constp.tile([P, DQ // P], fp32, name="bq_sb")
            bk_sb = constp.tile([P, DQ // P], fp32, name="bk_sb")
            nc.sync.dma_start(bq_sb[:], bq.rearrange("(o p) -> p o", p=P))
            nc.sync.dma_start(bk_sb[:], bk.rearrange("(o p) -> p o", p=P))
            bv1_bf = constp.tile([1, DV1], bf16, name="bv1_bf")
            nc.gpsimd.dma_start(bv1_bf[:], bv1[None, :])
            ones_bf = constp.tile([1, P], bf16, name="ones_bf")
            nc.gpsimd.memset(ones_bf[:], 1.0)

            # causal triangle mask (keep where t_local <= s_local)
            mask_f = constp.tile([P, P], fp32, name="mask_f")
            mask_bf = constp.tile([P, P], bf16, name="mask_bf")
            nc.gpsimd.memset(mask_f[:], 0.0)
            nc.gpsimd.affine_select(
                out=mask_f[:],
                in_=mask_f[:],
                compare_op=ALU.is_gt,  # iota > 0 ? keep in_ (0.0) : fill (1.0)
                fill=1.0,
                base=0,
                pattern=[[-1, P]],  # iota[p, f] = p - f;  p<=f -> fill=1.0
                channel_multiplier=1,
            )
            nc.vector.tensor_copy(mask_bf[:], mask_f[:])

            xT = big.tile([P, ND, S], bf16, name="xT")
            qT = big.tile([P, DQ // P, S], bf16, name="qT")
            kT = big.tile([P, DQ // P, S], bf16, name="kT")
            v1 = big.tile([P, NT, DV1], bf16, name="v1")
            # normalized z, two heads stacked per partition tile (for out-proj)
            zT2 = big.tile([P, DQ // P, S], bf16, name="zT2")

            # ---- emission helpers (units = closures emitted round-robin) --
            def emit_xload(c):
                xbf = dram.tile([SC, EMBED_DIM], bf16, name=f"xbf_{c}")
                nc.gpsimd.dma_start(xbf[:], x[c * SC : (c + 1) * SC, :])
                for d in range(ND):
                    nc.sync.dma_start(
                        xT[:, d, c * SC : (c + 1) * SC],
                        xbf[:, d * P : (d + 1) * P],
                        transpose=True,
                    )

            def unit_qk_proj(c, which, j):
                w_sb, b_sb, dstT = (
                    (wq_sb, bq_sb, qT) if which == "q" else (wk_sb, bk_sb, kT)
                )
                ps = psA.tile([P, SC], fp32, name="mmps")
                for d in range(ND):
                    nc.tensor.matmul(
                        ps[:],
                        w_sb[:, d, j * P : (j + 1) * P],
                        xT[:, d, c * SC : (c + 1) * SC],
                        start=(d == 0),
                        stop=(d == ND - 1),
                    )
                nc.vector.tensor_scalar(
                    dstT[:, j, c * SC : (c + 1) * SC],
                    ps[:],
                    b_sb[:, j : j + 1],
                    None,
                    ALU.add,
                )

            def unit_v_proj(c, tt):
                ps = psA.tile([P, SC], fp32, name="mmps")[:, :DV1]
                for d in range(ND):
                    nc.tensor.matmul(
                        ps[:],
                        xT[:, d, tt * P : (tt + 1) * P],
                        wv1_sb[:, d, :],
                        start=(d == 0),
                        stop=False,
                    )
                nc.tensor.matmul(ps[:], ones_bf[:], bv1_bf[:], start=False, stop=True)
                nc.vector.tensor_copy(v1[:, tt, :], ps[:])

            def proj_units(c):
                yield lambda: unit_qk_proj(c, "q", 0)
                yield lambda: unit_qk_proj(c, "k", 0)
                yield lambda: unit_qk_proj(c, "q", 1)
                yield lambda: unit_qk_proj(c, "k", 1)
                for tt in range(c * TPC, (c + 1) * TPC):
                    yield lambda tt=tt: unit_v_proj(c, tt)

            def unit_outproj(c, i):
                # out rows [i*P:(i+1)*P] = z_local.T @ wot  (i is a global s-tile)
                ps = [psA.tile([P, SC], fp32, name="mmps") for _ in range(2)]
                for n in range(2):
                    for j in range(DQ // P):
                        nc.tensor.matmul(
                            ps[n][:],
                            zT2[:, j, i * P : (i + 1) * P],
                            wot_sb[:, j, n * SC : (n + 1) * SC],
                            start=(j == 0),
                            stop=(j == DQ // P - 1),
                        )
                osb = outsb.tile([P, EMBED_DIM], fp32, name="osb")
                for n in range(2):
                    nc.vector.tensor_copy(osb[:, n * SC : (n + 1) * SC], ps[n][:])
                nc.sync.dma_start(out[i * P : (i + 1) * P, :], osb[:])

            def outproj_units(c):
                for i in range(c * TPC, (c + 1) * TPC):
                    yield lambda i=i: unit_outproj(c, i)

            # ---- attention for one (s-chunk, head-pair) ------------------
            def emit_attention(c, hp, filler):
                """filler: iterator of pending unit closures to interleave."""
                zps = [psZ.tile([VW, SC], fp32, name="zps") for _ in range(2)]
                ntt = (c + 1) * TPC
                for t0 in range(0, ntt, TGROUP):
                    tg = range(t0, min(t0 + TGROUP, ntt))
                    exs = {}
                    for tt in tg:
                        off = max(0, (tt - c * TPC)) * P
                        lg = psLG.tile([P, 2, SC], fp32, name="lgps")
                        ex = expp.tile([P, 2, SC], bf16, name="expt")
                        exs[tt] = ex
                        for i in range(2):
                            p0 = 64 * i
                            nc.tensor.matmul(
                                lg[:, i, off:],
                                kT[p0 : p0 + 64, hp, tt * P : (tt + 1) * P],
                                qT[p0 : p0 + 64, hp, c * SC + off : (c + 1) * SC],
                                start=True,
                                stop=True,
                                tile_position=(p0, 0),
                            )
                        nc.scalar.activation(ex[:, :, off:], lg[:, :, off:], AF.Exp)
                        if tt >= c * TPC:  # diagonal tile: causal triangle
                            for i in range(2):
                                nc.vector.tensor_tensor(
                                    ex[:, i, off : off + P],
                                    ex[:, i, off : off + P],
                                    mask_bf[:],
                                    ALU.mult,
                                )
                    for tt in tg:
                        off = max(0, (tt - c * TPC)) * P
                        ex = exs[tt]
                        for i in range(2):
                            h = 2 * hp + i
                            nc.tensor.matmul(
                                zps[i][:, off:],
                                v1[:, tt, h * VW : (h + 1) * VW],
                                ex[:, i, off:],
                                start=(tt == 0),
                                stop=(tt == ntt - 1),
                            )
                    for f in filler:  # fill PE slack with independent work
                        f()
                        break
                # normalization: z = zT_aug[0:64] * (1 / denom_row)
                for i in range(2):
                    h = 2 * hp + i
                    recip = small.tile([1, SC], fp32, name="recip")
                    rb = small.tile([HEAD_DIM, SC], fp32, name="recip_bc")
                    nc.vector.reciprocal(recip[:], zps[i][VW - 1 : VW, :])
                    nc.gpsimd.partition_broadcast(rb[:], recip[:])
                    if i == 0:  # even head: write partitions 0:64 directly
                        nc.vector.tensor_tensor(
                            zT2[:HEAD_DIM, hp, c * SC : (c + 1) * SC],
                            zps[i][:HEAD_DIM, :],
                            rb[:],
                            ALU.mult,
                        )
                    else:  # odd head: normalize to scratch, DMA-shift partitions
                        zodd = small.tile([HEAD_DIM, SC], bf16, name="zodd")
                        nc.vector.tensor_tensor(
                            zodd[:], zps[i][:HEAD_DIM, :], rb[:], ALU.mult
                        )
                        nc.sync.dma_start(
                            zT2[HEAD_DIM:P, hp, c * SC : (c + 1) * SC], zodd[:]
                        )

            # ---- main schedule -------------------------------------------
            emit_xload(0)
            for u in proj_units(0):
                u()
            pending = []
            for c in range(NSC):
                if c + 1 < NSC:
                    emit_xload(c + 1)
                    pending += list(proj_units(c + 1))
                if c >= 1:
                    pending += list(outproj_units(c - 1))
                filler = iter(pending)
                for hp in range(HEADS_PER_CORE // 2):
                    emit_attention(c, hp, filler)
                pending = list(filler)  # leftovers roll over
                for f in pending:
                    f()
                pending = []
            for u in outproj_units(NSC - 1):
                u()

    nc.finalize()
    return nc


def _get_nc(seq):
    if seq not in _NC_CACHE:
        _NC_CACHE[seq] = _build_nc(seq)
    return _NC_CACHE[seq]


def shard_inputs(x, Wq, bq, Wk, bk, Wv, bv, Wo):
    """Build the 8 per-core input maps (host-side sharding)."""
    scale = 1.0 / np.sqrt(np.float32(EMBED_DIM))
    x = np.asarray(x, np.float32)
    in_maps = []
    for c in range(N_CORES):
        b, g = divmod(c, CORES_PER_BATCH)
        sl = slice(g * DQ, (g + 1) * DQ)
        wv1 = np.zeros((EMBED_DIM, DV1), np.float32)
        bv1 = np.zeros((DV1,), np.float32)
        for h in range(HEADS_PER_CORE):
            col = g * DQ + h * HEAD_DIM
            wv1[:, h * VW : h * VW + HEAD_DIM] = Wv[:, col : col + HEAD_DIM]
            bv1[h * VW : h * VW + HEAD_DIM] = bv[col : col + HEAD_DIM]
            bv1[h * VW + HEAD_DIM] = 1.0
        # z columns per core are ordered [h0,h1 | h2,h3] pairwise with the odd
        # head in partitions 64:128 -> matches contiguous dv slice order.
        in_maps.append(
            {
                "x": np.ascontiguousarray(x[b]),
                "wq": np.ascontiguousarray(Wq[:, sl]) * scale,
                "bq": np.ascontiguousarray(bq[sl]) * scale,
                "wk": np.ascontiguousarray(Wk[:, sl]),
                "bk": np.ascontiguousarray(bk[sl]),
                "wv1": wv1,
                "bv1": bv1,
                "wot": np.ascontiguousarray(Wo[:, sl].T),
            }
        )
    return in_maps


def kernel(x, Wq, bq, Wk, bk, Wv, bv, Wo):
    from concourse.bass_utils import run_bass_kernel_spmd

    x = np.asarray(x, np.float32)
    B, S, D = x.shape
    nc = _get_nc(S)
    in_maps = shard_inputs(x, Wq, bq, Wk, bk, Wv, bv, Wo)
    res = run_bass_kernel_spmd(nc, in_maps, core_ids=list(range(N_CORES)))
    out = np.zeros((B, S, D), np.float32)
    for c in range(N_CORES):
        b = c // CORES_PER_BATCH
        out[b] += res.results[c]["out"]
    return out


# revision 32
# speedup vs baseline: 1.9902x; 1.0245x over previous
"""Causal self-attention (B=2, S=2048, D=1024, H=16) on 8 Trainium2 NeuronCores.

Sharding: core c handles batch b = c//4 and head-group g = c%4 (4 heads, 256
channels).  Per-core device program (identical NEFF on all cores):

  1. x[b] is DMA-cast f32->bf16 to a DRAM bounce per 512-row s-chunk, then
     xbar-transpose-loaded into SBUF as xT [D=8x128, S] (TensorE contracts
     over the partition axis, so activations must be D-major).
  2. QKV projections produce qT/kT [256, S] (head-dim on partitions) and
     v1 [S, 260] (natural orientation, with a ones-column per head appended
     via the bias row so the PV matmul also yields softmax denominators).
     The softmax 1/sqrt(D) scale is folded into Wq/bq on the host.
  3. Attention per head pair: logitsT[t, s] tiles from K=64 matmuls with two
     heads packed in the PE array via row tile_position into one 2-bank PSUM
     tile, exp on ScalarE straight out of PSUM, causal triangle mask as a
     bf16 multiply on diagonal tiles, PV accumulates
     zT_aug[65, s] = [v.T @ expT ; sum_t expT] (row 64 = denominator).
     The t-loop is emitted in groups (all QK/exp of a group, then PV) so the
     in-order PE never waits on ScalarE; projection work for the next s-chunk
     is interleaved between groups to fill PE slack.
  4. Each core computes its full-width partial output
     out_partial = z_local @ Wo.T[local 256 rows, :], pipelined per s-chunk.
     The host sums the 4 partials per batch (the row-parallel reduction).
"""

import numpy as np

EMBED_DIM = 1024
NUM_HEADS = 16
HEAD_DIM = 64
BATCH = 2
N_CORES = 8
CORES_PER_BATCH = 4
HEADS_PER_CORE = 4
DQ = HEADS_PER_CORE * HEAD_DIM  # 256 q/k/v channels per core
VW = HEAD_DIM + 1  # v block width incl. ones column
DV1 = HEADS_PER_CORE * VW  # 260
P = 128

_NC_CACHE = {}


def _build_nc(seq):
    import concourse.bass as bass  # noqa: F401
    import concourse.mybir as mybir
    import concourse.tile as tile
    from concourse import bacc

    fp32 = mybir.dt.float32
    bf16 = mybir.dt.bfloat16
    AF = mybir.ActivationFunctionType
    ALU = mybir.AluOpType

    S = seq
    SC = 512  # s-chunk width
    NSC = S // SC  # s-chunks
    NT = S // P  # t-tiles
    ND = EMBED_DIM // P  # D-tiles (8)
    TPC = SC // P  # t-tiles per s-chunk (4)
    TGROUP = 8  # attention t-loop software-pipeline group

    nc = bacc.Bacc("TRN2", target_bir_lowering=False, num_devices=N_CORES)

    x = nc.declare_dram_parameter("x", [S, EMBED_DIM], fp32, isOutput=False)
    wq = nc.declare_dram_parameter("wq", [EMBED_DIM, DQ], bf16, isOutput=False)
    bq = nc.declare_dram_parameter("bq", [DQ], fp32, isOutput=False)
    wk = nc.declare_dram_parameter("wk", [EMBED_DIM, DQ], bf16, isOutput=False)
    bk = nc.declare_dram_parameter("bk", [DQ], fp32, isOutput=False)
    wv1 = nc.declare_dram_parameter("wv1", [EMBED_DIM, DV1], bf16, isOutput=False)
    bv1 = nc.declare_dram_parameter("bv1", [DV1], bf16, isOutput=False)
    wot = nc.declare_dram_parameter("wot", [DQ, EMBED_DIM], bf16, isOutput=False)
    out = nc.declare_dram_parameter("out", [S, EMBED_DIM], fp32, isOutput=True)

    with tile.TileContext(nc) as tc:
        with (
            tc.tile_pool(name="const", bufs=1) as constp,
            tc.tile_pool(name="dram", bufs=1, space="DRAM") as dram,
            tc.tile_pool(name="big", bufs=1) as big,
            tc.tile_pool(name="exp", bufs=12) as expp,
            tc.tile_pool(name="small", bufs=4) as small,
            tc.tile_pool(name="outsb", bufs=3) as outsb,
            tc.tile_pool(name="xnat", bufs=2) as xnp,
            tc.tile_pool(name="psA", bufs=2, space="PSUM") as psA,
            tc.tile_pool(name="psLG", bufs=2, space="PSUM") as psLG,
            tc.tile_pool(name="psZ", bufs=2, space="PSUM") as psZ,
        ):
            # ---- tiles ----------------------------------------------------
            wq_sb = big.tile([P, ND, DQ], bf16, name="wq_sb")
            wk_sb = big.tile([P, ND, DQ], bf16, name="wk_sb")
            wv1_sb = big.tile([P, ND, DV1], bf16, name="wv1_sb")
            wot_sb = big.tile([P, DQ // P, EMBED_DIM], bf16, name="wot_sb")
            bq_sb = constp.tile([P, DQ // P], fp32, name="bq_sb")
            bk_sb = constp.tile([P, DQ // P], fp32, name="bk_sb")
            bv1_bf = constp.tile([1, DV1], bf16, name="bv1_bf")
            ones_bf = constp.tile([1, P], bf16, name="ones_bf")
            mask_f = constp.tile([P, P], fp32, name="mask_f")
            mask_bf = constp.tile([P, P], bf16, name="mask_bf")
            ident_f = constp.tile([P, P], fp32, name="ident_f")
            ident_bf = constp.tile([P, P], bf16, name="ident_bf")
            xT = big.tile([P, ND, S], bf16, name="xT")
            qT = big.tile([P, DQ // P, S], bf16, name="qT")
            kT = big.tile([P, DQ // P, S], bf16, name="kT")
            v1 = big.tile([P, NT, DV1], bf16, name="v1")
            # normalized z, two heads stacked per partition tile (for out-proj)
            zT2 = big.tile([P, DQ // P, S], bf16, name="zT2")

            # ---- emission helpers (units = closures emitted round-robin) --
            def emit_xload(c):
                # SWDGE-cast x rows straight into SBUF (f32->bf16), then
                # transpose 128x128 blocks on the PE (identity stays moving,
                # x block is the stationary operand; bf16 transpose-mode runs
                # at 1 cycle/row).  Avoids the DMA-xbar transpose path, whose
                # mode-switch hazard serializes against all other DMA traffic.
                xn = xnp.tile([P, TPC, EMBED_DIM], bf16, name="xnat")
                nc.gpsimd.dma_start(
                    xn[:],
                    x[c * SC : (c + 1) * SC, :].rearrange("(o p) n -> p o n", p=P),
                )
                for d in range(ND):
                    ps = psA.tile([P, SC], bf16, name="mmps")
                    for tt in range(TPC):
                        nc.tensor.transpose(
                            ps[:, tt * P : (tt + 1) * P],
                            xn[:, tt, d * P : (d + 1) * P],
                            ident_bf[:],
                        )
                    nc.vector.tensor_copy(xT[:, d, c * SC : (c + 1) * SC], ps[:])

            def unit_qk_proj(c, which, j, ps=None, dr=None):
                w_sb, b_sb, dstT = (
                    (wq_sb, bq_sb, qT) if which == "q" else (wk_sb, bk_sb, kT)
                )
                if ps is None:
                    ps = psA.tile([P, SC], fp32, name="mmps")
                for d in dr if dr is not None else range(ND):
                    nc.tensor.matmul(
                        ps[:],
                        w_sb[:, d, j * P : (j + 1) * P],
                        xT[:, d, c * SC : (c + 1) * SC],
                        start=(d == 0),
                        stop=(d == ND - 1),
                    )
                if dr is None or dr[-1] == ND - 1:
                    nc.vector.tensor_scalar(
                        dstT[:, j, c * SC : (c + 1) * SC],
                        ps[:],
                        b_sb[:, j : j + 1],
                        None,
                        ALU.add,
                    )
                return ps

            def unit_v_proj(c, tt):
                ps = psA.tile([P, SC], fp32, name="mmps")[:, :DV1]
                for d in range(ND):
                    nc.tensor.matmul(
                        ps[:],
                        xT[:, d, tt * P : (tt + 1) * P],
                        wv1_sb[:, d, :],
                        start=(d == 0),
                        stop=False,
                    )
                nc.tensor.matmul(ps[:], ones_bf[:], bv1_bf[:], start=False, stop=True)
                nc.vector.tensor_copy(v1[:, tt, :], ps[:])

            def proj_units(c):
                yield lambda: unit_qk_proj(c, "q", 0)
                yield lambda: unit_qk_proj(c, "k", 0)
                yield lambda: unit_qk_proj(c, "q", 1)
                yield lambda: unit_qk_proj(c, "k", 1)
                for tt in range(c * TPC, (c + 1) * TPC):
                    yield lambda tt=tt: unit_v_proj(c, tt)

            def unit_outproj(c, i):
                # out rows [i*P:(i+1)*P] = z_local.T @ wot  (i is a global s-tile)
                ps = [psA.tile([P, SC], fp32, name="mmps") for _ in range(2)]
                for n in range(2):
                    for j in range(DQ // P):
                        nc.tensor.matmul(
                            ps[n][:],
                            zT2[:, j, i * P : (i + 1) * P],
                            wot_sb[:, j, n * SC : (n + 1) * SC],
                            start=(j == 0),
                            stop=(j == DQ // P - 1),
                        )
                osb = outsb.tile([P, EMBED_DIM], fp32, name="osb")
                for n in range(2):
                    nc.vector.tensor_copy(osb[:, n * SC : (n + 1) * SC], ps[n][:])
                nc.sync.dma_start(out[i * P : (i + 1) * P, :], osb[:])

            def outproj_units(c):
                for i in range(c * TPC, (c + 1) * TPC):
                    yield lambda i=i: unit_outproj(c, i)

            # ---- attention for one (s-chunk, head-pair) ------------------
            def emit_attention(c, hp, filler):
                """Rolling QK -> exp -> (lag-2) PV pipeline; `filler` units are
                popped periodically to fill the PE's exp-wait slack."""
                LAG = 2
                zps = [psZ.tile([VW, SC], fp32, name="zps") for _ in range(2)]
                ntt = (c + 1) * TPC
                exs = {}

                def emit_pv(tt):
                    off = max(0, (tt - c * TPC)) * P
                    ex = exs.pop(tt)
                    for i in range(2):
                        h = 2 * hp + i
                        nc.tensor.matmul(
                            zps[i][:, off:],
                            v1[:, tt, h * VW : (h + 1) * VW],
                            ex[:, i, off:],
                            start=(tt == 0),
                            stop=(tt == ntt - 1),
                        )

                for tt in range(ntt):
                    off = max(0, (tt - c * TPC)) * P
                    lg = psLG.tile([P, 2, SC], fp32, name="lgps")
                    ex = expp.tile([P, 2, SC], bf16, name="expt")
                    exs[tt] = ex
                    for i in range(2):
                        p0 = 64 * i
                        nc.tensor.matmul(
                            lg[:, i, off:],
                            kT[p0 : p0 + 64, hp, tt * P : (tt + 1) * P],
                            qT[p0 : p0 + 64, hp, c * SC + off : (c + 1) * SC],
                            start=True,
                            stop=True,
                            tile_position=(p0, 0),
                        )
                    nc.scalar.activation(ex[:, :, off:], lg[:, :, off:], AF.Exp)
                    if tt >= c * TPC:  # diagonal tile: causal triangle
                        for i in range(2):
                            nc.vector.tensor_tensor(
                                ex[:, i, off : off + P],
                                ex[:, i, off : off + P],
                                mask_bf[:],
                                ALU.mult,
                            )
                    if tt >= LAG:
                        emit_pv(tt - LAG)
                    if tt % 4 == 3:
                        for f in filler:  # fill PE exp-wait slack
                            f()
                            break
                for tt in range(max(0, ntt - LAG), ntt):
                    emit_pv(tt)
                # normalization: z = zT_aug[0:64] * (1 / denom_row)
                for i in range(2):
                    h = 2 * hp + i
                    recip = small.tile([1, SC], fp32, name="recip")
                    rb = small.tile([HEAD_DIM, SC], fp32, name="recip_bc")
                    nc.vector.reciprocal(recip[:], zps[i][VW - 1 : VW, :])
                    nc.gpsimd.partition_broadcast(rb[:], recip[:])
                    if i == 0:  # even head: write partitions 0:64 directly
                        nc.vector.tensor_tensor(
                            zT2[:HEAD_DIM, hp, c * SC : (c + 1) * SC],
                            zps[i][:HEAD_DIM, :],
                            rb[:],
                            ALU.mult,
                        )
                    else:  # odd head: normalize to scratch, DMA-shift partitions
                        zodd = small.tile([HEAD_DIM, SC], bf16, name="zodd")
                        nc.vector.tensor_tensor(
                            zodd[:], zps[i][:HEAD_DIM, :], rb[:], ALU.mult
                        )
                        nc.sync.dma_start(
                            zT2[HEAD_DIM:P, hp, c * SC : (c + 1) * SC], zodd[:]
                        )

            # ---- main schedule -------------------------------------------
            # ---- startup: constants first (identity feeds the transposes) -
            nc.gpsimd.memset(ones_bf[:], 1.0)
            # causal triangle mask (keep where t_local <= s_local)
            nc.gpsimd.memset(mask_f[:], 0.0)
            nc.gpsimd.affine_select(
                out=mask_f[:],
                in_=mask_f[:],
                compare_op=ALU.is_gt,  # iota > 0 ? keep in_ (0.0) : fill (1.0)
                fill=1.0,
                base=0,
                pattern=[[-1, P]],  # iota[p, f] = p - f;  p<=f -> fill=1.0
                channel_multiplier=1,
            )
            nc.vector.tensor_copy(mask_bf[:], mask_f[:])
            from concourse.masks import make_identity

            make_identity(nc, ident_f[:])
            nc.vector.tensor_copy(ident_bf[:], ident_f[:])

            emit_xload(0)
            nc.sync.dma_start(wq_sb[:], wq.rearrange("(o p) n -> p o n", p=P))
            nc.sync.dma_start(bq_sb[:], bq.rearrange("(o p) -> p o", p=P))
            nc.sync.dma_start(wk_sb[:], wk.rearrange("(o p) n -> p o n", p=P))
            nc.sync.dma_start(bk_sb[:], bk.rearrange("(o p) -> p o", p=P))
            nc.sync.dma_start(wv1_sb[:], wv1.rearrange("(o p) n -> p o n", p=P))
            nc.sync.dma_start(bv1_bf[:], bv1[None, :])
            nc.sync.dma_start(wot_sb[:], wot.rearrange("(o p) n -> p o n", p=P))
            # chunk 0: split the first projection into d-halves so the first
            # matmuls only wait for the first half of the x transposes
            ps0 = unit_qk_proj(0, "q", 0, dr=range(ND // 2))
            unit_qk_proj(0, "q", 0, ps=ps0, dr=range(ND // 2, ND))
            unit_qk_proj(0, "k", 0)
            unit_qk_proj(0, "q", 1)
            unit_qk_proj(0, "k", 1)
            for tt in range(TPC):
                unit_v_proj(0, tt)
            for c in range(NSC):
                # in-group fillers: only always-ready work (prev chunk outproj)
                pending = list(outproj_units(c - 1)) if c >= 1 else []
                filler = iter(pending)
                emit_attention(c, 0, filler)
                if c + 1 < NSC:
                    emit_xload(c + 1)
                emit_attention(c, 1, filler)
                for f in filler:
                    f()
                # then next chunk's projections
                if c + 1 < NSC:
                    for u in proj_units(c + 1):
                        u()
            for u in outproj_units(NSC - 1):
                u()

    nc.finalize()
    return nc


def _get_nc(seq):
    if seq not in _NC_CACHE:
        _NC_CACHE[seq] = _build_nc(seq)
    return _NC_CACHE[seq]


def shard_inputs(x, Wq, bq, Wk, bk, Wv, bv, Wo):
    """Build the 8 per-core input maps (host-side sharding)."""
    import ml_dtypes

    bf = ml_dtypes.bfloat16
    scale = 1.0 / np.sqrt(np.float32(EMBED_DIM))
    x = np.asarray(x, np.float32)
    in_maps = []
    for c in range(N_CORES):
        b, g = divmod(c, CORES_PER_BATCH)
        sl = slice(g * DQ, (g + 1) * DQ)
        wv1 = np.zeros((EMBED_DIM, DV1), np.float32)
        bv1 = np.zeros((DV1,), np.float32)
        for h in range(HEADS_PER_CORE):
            col = g * DQ + h * HEAD_DIM
            wv1[:, h * VW : h * VW + HEAD_DIM] = Wv[:, col : col + HEAD_DIM]
            bv1[h * VW : h * VW + HEAD_DIM] = bv[col : col + HEAD_DIM]
            bv1[h * VW + HEAD_DIM] = 1.0
        in_maps.append(
            {
                "x": np.ascontiguousarray(x[b]),
                "wq": (np.ascontiguousarray(Wq[:, sl]) * scale).astype(bf),
                "bq": np.ascontiguousarray(bq[sl]) * scale,
                "wk": np.ascontiguousarray(Wk[:, sl]).astype(bf),
                "bk": np.ascontiguousarray(bk[sl]),
                "wv1": wv1.astype(bf),
                "bv1": bv1.astype(bf),
                "wot": np.ascontiguousarray(Wo[:, sl].T).astype(bf),
            }
        )
    return in_maps


def kernel(x, Wq, bq, Wk, bk, Wv, bv, Wo):
    from concourse.bass_utils import run_bass_kernel_spmd

    x = np.asarray(x, np.float32)
    B, S, D = x.shape
    nc = _get_nc(S)
    in_maps = shard_inputs(x, Wq, bq, Wk, bk, Wv, bv, Wo)
    res = run_bass_kernel_spmd(nc, in_maps, core_ids=list(range(N_CORES)))
    out = np.zeros((B, S, D), np.float32)
    for c in range(N_CORES):
        b = c // CORES_PER_BATCH
        out[b] += res.results[c]["out"]
    return out


# revision 38
# speedup vs baseline: 2.0029x; 1.0064x over previous
"""Causal self-attention (B=2, S=2048, D=1024, H=16) on 8 Trainium2 NeuronCores.

Sharding: core c handles batch b = c//4 and head-group g = c%4 (4 heads, 256
channels).  Per-core device program (identical NEFF on all cores):

  1. x[b] is DMA-cast f32->bf16 to a DRAM bounce per 512-row s-chunk, then
     xbar-transpose-loaded into SBUF as xT [D=8x128, S] (TensorE contracts
     over the partition axis, so activations must be D-major).
  2. QKV projections produce qT/kT [256, S] (head-dim on partitions) and
     v1 [S, 260] (natural orientation, with a ones-column per head appended
     via the bias row so the PV matmul also yields softmax denominators).
     The softmax 1/sqrt(D) scale is folded into Wq/bq on the host.
  3. Attention per head pair: logitsT[t, s] tiles from K=64 matmuls with two
     heads packed in the PE array via row tile_position into one 2-bank PSUM
     tile, exp on ScalarE straight out of PSUM, causal triangle mask as a
     bf16 multiply on diagonal tiles, PV accumulates
     zT_aug[65, s] = [v.T @ expT ; sum_t expT] (row 64 = denominator).
     The t-loop is emitted in groups (all QK/exp of a group, then PV) so the
     in-order PE never waits on ScalarE; projection work for the next s-chunk
     is interleaved between groups to fill PE slack.
  4. Each core computes its full-width partial output
     out_partial = z_local @ Wo.T[local 256 rows, :], pipelined per s-chunk.
     The host sums the 4 partials per batch (the row-parallel reduction).
"""

import numpy as np

EMBED_DIM = 1024
NUM_HEADS = 16
HEAD_DIM = 64
BATCH = 2
N_CORES = 8
CORES_PER_BATCH = 4
HEADS_PER_CORE = 4
DQ = HEADS_PER_CORE * HEAD_DIM  # 256 q/k/v channels per core
VW = HEAD_DIM + 1  # v block width incl. ones column
DV1 = HEADS_PER_CORE * VW  # 260
P = 128

_NC_CACHE = {}


def _build_nc(seq):
    import concourse.bass as bass  # noqa: F401
    import concourse.mybir as mybir
    import concourse.tile as tile
    from concourse import bacc

    fp32 = mybir.dt.float32
    bf16 = mybir.dt.bfloat16
    AF = mybir.ActivationFunctionType
    ALU = mybir.AluOpType

    S = seq
    SC = 512  # s-chunk width
    NSC = S // SC  # s-chunks
    NT = S // P  # t-tiles
    ND = EMBED_DIM // P  # D-tiles (8)
    TPC = SC // P  # t-tiles per s-chunk (4)

    nc = bacc.Bacc("TRN2", target_bir_lowering=False, num_devices=N_CORES)

    x = nc.declare_dram_parameter("x", [S, EMBED_DIM], fp32, isOutput=False)
    wq = nc.declare_dram_parameter("wq", [EMBED_DIM, DQ], bf16, isOutput=False)
    bq = nc.declare_dram_parameter("bq", [DQ], fp32, isOutput=False)
    wk = nc.declare_dram_parameter("wk", [EMBED_DIM, DQ], bf16, isOutput=False)
    bk = nc.declare_dram_parameter("bk", [DQ], fp32, isOutput=False)
    wv1 = nc.declare_dram_parameter("wv1", [EMBED_DIM, DV1], bf16, isOutput=False)
    bv1 = nc.declare_dram_parameter("bv1", [DV1], bf16, isOutput=False)
    wot = nc.declare_dram_parameter("wot", [DQ, EMBED_DIM], bf16, isOutput=False)
    out = nc.declare_dram_parameter("out", [S, EMBED_DIM], fp32, isOutput=True)

    with tile.TileContext(nc) as tc:
        with (
            tc.tile_pool(name="const", bufs=1) as constp,
            tc.tile_pool(name="big", bufs=1) as big,
            tc.tile_pool(name="exp", bufs=16) as expp,
            tc.tile_pool(name="small", bufs=4) as small,
            tc.tile_pool(name="outsb", bufs=3) as outsb,
            tc.tile_pool(name="xnat", bufs=2) as xnp,
            tc.tile_pool(name="psA", bufs=2, space="PSUM") as psA,
            tc.tile_pool(name="psLG", bufs=2, space="PSUM") as psLG,
            tc.tile_pool(name="psZ", bufs=2, space="PSUM") as psZ,
        ):
            # ---- tiles ----------------------------------------------------
            wq_sb = big.tile([P, ND, DQ], bf16, name="wq_sb")
            wk_sb = big.tile([P, ND, DQ], bf16, name="wk_sb")
            wv1_sb = big.tile([P, ND, DV1], bf16, name="wv1_sb")
            wot_sb = big.tile([P, DQ // P, EMBED_DIM], bf16, name="wot_sb")
            bq_sb = constp.tile([P, DQ // P], fp32, name="bq_sb")
            bk_sb = constp.tile([P, DQ // P], fp32, name="bk_sb")
            bv1_bf = constp.tile([1, DV1], bf16, name="bv1_bf")
            ones_bf = constp.tile([1, P], bf16, name="ones_bf")
            mask_f = constp.tile([P, P], fp32, name="mask_f")
            mask_bf = constp.tile([P, P], bf16, name="mask_bf")
            ident_f = constp.tile([P, P], fp32, name="ident_f")
            ident_bf = constp.tile([P, P], bf16, name="ident_bf")
            xT = big.tile([P, ND, S], bf16, name="xT")
            qT = big.tile([P, DQ // P, S], bf16, name="qT")
            kT = big.tile([P, DQ // P, S], bf16, name="kT")
            v1 = big.tile([P, NT, DV1], bf16, name="v1")
            # normalized z, two heads stacked per partition tile (for out-proj)
            zT2 = big.tile([P, DQ // P, S], bf16, name="zT2")

            # ---- emission helpers (units = closures emitted round-robin) --
            def emit_xload(c):
                # SWDGE-cast x rows straight into SBUF (f32->bf16), then
                # transpose 128x128 blocks on the PE (identity stays moving,
                # x block is the stationary operand; bf16 transpose-mode runs
                # at 1 cycle/row).  Avoids the DMA-xbar transpose path, whose
                # mode-switch hazard serializes against all other DMA traffic.
                xn = xnp.tile([P, TPC, EMBED_DIM], bf16, name="xnat")
                nc.gpsimd.dma_start(
                    xn[:],
                    x[c * SC : (c + 1) * SC, :].rearrange("(o p) n -> p o n", p=P),
                )
                for d in range(ND):
                    ps = psA.tile([P, SC], bf16, name="mmps")
                    for tt in range(TPC):
                        nc.tensor.transpose(
                            ps[:, tt * P : (tt + 1) * P],
                            xn[:, tt, d * P : (d + 1) * P],
                            ident_bf[:],
                        )
                    nc.vector.tensor_copy(xT[:, d, c * SC : (c + 1) * SC], ps[:])

            def unit_qk_proj(c, which, j, ps=None, dr=None):
                w_sb, b_sb, dstT = (
                    (wq_sb, bq_sb, qT) if which == "q" else (wk_sb, bk_sb, kT)
                )
                if ps is None:
                    ps = psA.tile([P, SC], fp32, name="mmps")
                for d in dr if dr is not None else range(ND):
                    nc.tensor.matmul(
                        ps[:],
                        w_sb[:, d, j * P : (j + 1) * P],
                        xT[:, d, c * SC : (c + 1) * SC],
                        start=(d == 0),
                        stop=(d == ND - 1),
                    )
                if dr is None or dr[-1] == ND - 1:
                    nc.vector.tensor_scalar(
                        dstT[:, j, c * SC : (c + 1) * SC],
                        ps[:],
                        b_sb[:, j : j + 1],
                        None,
                        ALU.add,
                    )
                return ps

            def unit_v_proj(c, tt):
                ps = psA.tile([P, SC], fp32, name="mmps")[:, :DV1]
                for d in range(ND):
                    nc.tensor.matmul(
                        ps[:],
                        xT[:, d, tt * P : (tt + 1) * P],
                        wv1_sb[:, d, :],
                        start=(d == 0),
                        stop=False,
                    )
                nc.tensor.matmul(ps[:], ones_bf[:], bv1_bf[:], start=False, stop=True)
                nc.vector.tensor_copy(v1[:, tt, :], ps[:])

            def proj_units(c):
                yield lambda: unit_qk_proj(c, "q", 0)
                yield lambda: unit_qk_proj(c, "k", 0)
                yield lambda: unit_qk_proj(c, "q", 1)
                yield lambda: unit_qk_proj(c, "k", 1)
                for tt in range(c * TPC, (c + 1) * TPC):
                    yield lambda tt=tt: unit_v_proj(c, tt)

            def unit_outproj(c, i):
                # out rows [i*P:(i+1)*P] = z_local.T @ wot  (i is a global s-tile)
                ps = [psA.tile([P, SC], fp32, name="mmps") for _ in range(2)]
                for n in range(2):
                    for j in range(DQ // P):
                        nc.tensor.matmul(
                            ps[n][:],
                            zT2[:, j, i * P : (i + 1) * P],
                            wot_sb[:, j, n * SC : (n + 1) * SC],
                            start=(j == 0),
                            stop=(j == DQ // P - 1),
                        )
                osb = outsb.tile([P, EMBED_DIM], fp32, name="osb")
                for n in range(2):
                    nc.vector.tensor_copy(osb[:, n * SC : (n + 1) * SC], ps[n][:])
                nc.sync.dma_start(out[i * P : (i + 1) * P, :], osb[:])

            def outproj_units(c):
                for i in range(c * TPC, (c + 1) * TPC):
                    yield lambda i=i: unit_outproj(c, i)

            # ---- attention for one (s-chunk, head-pair) ------------------
            def emit_attention(c, hp, filler):
                """Rolling QK -> exp -> (lag-2) PV pipeline; `filler` units are
                popped periodically to fill the PE's exp-wait slack."""
                LAG = 3
                zps = [psZ.tile([VW, SC], fp32, name="zps") for _ in range(2)]
                ntt = (c + 1) * TPC
                exs = {}

                def emit_pv(tt):
                    off = max(0, (tt - c * TPC)) * P
                    ex = exs.pop(tt)
                    for i in range(2):
                        h = 2 * hp + i
                        nc.tensor.matmul(
                            zps[i][:, off:],
                            v1[:, tt, h * VW : (h + 1) * VW],
                            ex[:, i, off:],
                            start=(tt == 0),
                            stop=(tt == ntt - 1),
                        )

                for tt in range(ntt):
                    off = max(0, (tt - c * TPC)) * P
                    lg = psLG.tile([P, 2, SC], fp32, name="lgps")
                    ex = expp.tile([P, 2, SC], bf16, name="expt")
                    exs[tt] = ex
                    for i in range(2):
                        p0 = 64 * i
                        nc.tensor.matmul(
                            lg[:, i, off:],
                            kT[p0 : p0 + 64, hp, tt * P : (tt + 1) * P],
                            qT[p0 : p0 + 64, hp, c * SC + off : (c + 1) * SC],
                            start=True,
                            stop=True,
                            tile_position=(p0, 0),
                        )
                    nc.scalar.activation(ex[:, :, off:], lg[:, :, off:], AF.Exp)
                    if tt >= c * TPC:  # diagonal tile: causal triangle
                        for i in range(2):
                            nc.vector.tensor_tensor(
                                ex[:, i, off : off + P],
                                ex[:, i, off : off + P],
                                mask_bf[:],
                                ALU.mult,
                            )
                    if tt >= LAG:
                        emit_pv(tt - LAG)
                    if tt % 4 == 3:
                        for f in filler:  # fill PE exp-wait slack
                            f()
                            break
                for tt in range(max(0, ntt - LAG), ntt):
                    emit_pv(tt)
                # normalization: z = zT_aug[0:64] * (1 / denom_row)
                for i in range(2):
                    h = 2 * hp + i
                    recip = small.tile([1, SC], fp32, name="recip")
                    rb = small.tile([HEAD_DIM, SC], fp32, name="recip_bc")
                    nc.vector.reciprocal(recip[:], zps[i][VW - 1 : VW, :])
                    nc.gpsimd.partition_broadcast(rb[:], recip[:])
                    if i == 0:  # even head: write partitions 0:64 directly
                        nc.vector.tensor_tensor(
                            zT2[:HEAD_DIM, hp, c * SC : (c + 1) * SC],
                            zps[i][:HEAD_DIM, :],
                            rb[:],
                            ALU.mult,
                        )
                    else:  # odd head: normalize to scratch, DMA-shift partitions
                        zodd = small.tile([HEAD_DIM, SC], bf16, name="zodd")
                        nc.vector.tensor_tensor(
                            zodd[:], zps[i][:HEAD_DIM, :], rb[:], ALU.mult
                        )
                        nc.sync.dma_start(
                            zT2[HEAD_DIM:P, hp, c * SC : (c + 1) * SC], zodd[:]
                        )

            # ---- main schedule -------------------------------------------
            # ---- startup: constants first (identity feeds the transposes) -
            nc.gpsimd.memset(ones_bf[:], 1.0)
            # causal triangle mask (keep where t_local <= s_local)
            nc.gpsimd.memset(mask_f[:], 0.0)
            nc.gpsimd.affine_select(
                out=mask_f[:],
                in_=mask_f[:],
                compare_op=ALU.is_gt,  # iota > 0 ? keep in_ (0.0) : fill (1.0)
                fill=1.0,
                base=0,
                pattern=[[-1, P]],  # iota[p, f] = p - f;  p<=f -> fill=1.0
                channel_multiplier=1,
            )
            nc.vector.tensor_copy(mask_bf[:], mask_f[:])
            from concourse.masks import make_identity

            make_identity(nc, ident_f[:])
            nc.vector.tensor_copy(ident_bf[:], ident_f[:])

            emit_xload(0)
            nc.sync.dma_start(wq_sb[:], wq.rearrange("(o p) n -> p o n", p=P))
            nc.sync.dma_start(bq_sb[:], bq.rearrange("(o p) -> p o", p=P))
            nc.sync.dma_start(wk_sb[:], wk.rearrange("(o p) n -> p o n", p=P))
            nc.sync.dma_start(bk_sb[:], bk.rearrange("(o p) -> p o", p=P))
            nc.sync.dma_start(wv1_sb[:], wv1.rearrange("(o p) n -> p o n", p=P))
            nc.sync.dma_start(bv1_bf[:], bv1[None, :])
            nc.sync.dma_start(wot_sb[:], wot.rearrange("(o p) n -> p o n", p=P))
            # chunk 0: split the first projection into d-halves so the first
            # matmuls only wait for the first half of the x transposes
            ps0 = unit_qk_proj(0, "q", 0, dr=range(ND // 2))
            unit_qk_proj(0, "q", 0, ps=ps0, dr=range(ND // 2, ND))
            unit_qk_proj(0, "k", 0)
            unit_qk_proj(0, "q", 1)
            unit_qk_proj(0, "k", 1)
            for tt in range(TPC):
                unit_v_proj(0, tt)
            for c in range(NSC):
                # in-group fillers: only always-ready work (prev chunk outproj)
                pending = list(outproj_units(c - 1)) if c >= 1 else []
                filler = iter(pending)
                emit_attention(c, 0, filler)
                if c + 1 < NSC:
                    emit_xload(c + 1)
                emit_attention(c, 1, filler)
                for f in filler:
                    f()
                # then next chunk's projections
                if c + 1 < NSC:
                    for u in proj_units(c + 1):
                        u()
            for u in outproj_units(NSC - 1):
                u()

    nc.finalize()
    return nc


def _get_nc(seq):
    if seq not in _NC_CACHE:
        _NC_CACHE[seq] = _build_nc(seq)
    return _NC_CACHE[seq]


def shard_inputs(x, Wq, bq, Wk, bk, Wv, bv, Wo):
    """Build the 8 per-core input maps (host-side sharding)."""
    import ml_dtypes

    bf = ml_dtypes.bfloat16
    scale = 1.0 / np.sqrt(np.float32(EMBED_DIM))
    x = np.asarray(x, np.float32)
    in_maps = []
    for c in range(N_CORES):
        b, g = divmod(c, CORES_PER_BATCH)
        sl = slice(g * DQ, (g + 1) * DQ)
        wv1 = np.zeros((EMBED_DIM, DV1), np.float32)
        bv1 = np.zeros((DV1,), np.float32)
        for h in range(HEADS_PER_CORE):
            col = g * DQ + h * HEAD_DIM
            wv1[:, h * VW : h * VW + HEAD_DIM] = Wv[:, col : col + HEAD_DIM]
            bv1[h * VW : h * VW + HEAD_DIM] = bv[col : col + HEAD_DIM]
            bv1[h * VW + HEAD_DIM] = 1.0
        in_maps.append(
            {
                "x": np.ascontiguousarray(x[b]),
                "wq": (np.ascontiguousarray(Wq[:, sl]) * scale).astype(bf),
                "bq": np.ascontiguousarray(bq[sl]) * scale,
                "wk": np.ascontiguousarray(Wk[:, sl]).astype(bf),
                "bk": np.ascontiguousarray(bk[sl]),
                "wv1": wv1.astype(bf),
                "bv1": bv1.astype(bf),
                "wot": np.ascontiguousarray(Wo[:, sl].T).astype(bf),
            }
        )
    return in_maps


def kernel(x, Wq, bq, Wk, bk, Wv, bv, Wo):
    from concourse.bass_utils import run_bass_kernel_spmd

    x = np.asarray(x, np.float32)
    B, S, D = x.shape
    nc = _get_nc(S)
    in_maps = shard_inputs(x, Wq, bq, Wk, bk, Wv, bv, Wo)
    res = run_bass_kernel_spmd(nc, in_maps, core_ids=list(range(N_CORES)))
    out = np.zeros((B, S, D), np.float32)
    for c in range(N_CORES):
        b = c // CORES_PER_BATCH
        out[b] += res.results[c]["out"]
    return out
